# revision 1
# baseline (speedup 1.0000x reference)
"""Adaptive-softmax NLL loss kernel for 8 TRN2 NeuronCores.

Strategy (vocab-parallel tensor parallelism + cluster-sorted tokens):
  - Each core owns a 1/8 column slice of each cluster's vocab range
    (250 + 1000 + 5032 cols) plus the shared remainder column 50256
    (its exp is scaled by 1/8 on every core so the all-reduced sum is
    exact).
  - Tokens are host-sorted by cluster id so each 128-token tile is
    (almost always) single-cluster; pure tiles only compute their own
    cluster's vocab columns (~70% of the full matmul/exp work, since
    the reference's other-cluster log-softmaxes are masked out anyway).
    The output is unscrambled on the host.
  - Main logits matmul runs in fp8e4m3 with DoubleRow perf mode
    (K packed 2x per PE cell). Inputs are pre-scaled (x*16, w*64) to
    dodge fp8 subnormals; the 1/1024 descale is folded into the
    ScalarE exp's free affine (exp(scale*psum + bias)).
  - ScalarE computes exp over up to 2048-col PSUM spans with a fused
    free-dim accumulate, giving per-cluster partial sum-exp per token.
  - Target logit x[t] . w[y_t] comes from an indirect-DMA gather of
    the owned weight rows (bf16, transposed shard) + multiply/reduce
    on VectorE, masked by ownership.
  - Two 32KB AllReduces (token halves) combine (S0, S1, S2, tgt); the
    first is issued halfway through the last column group so it hides
    under compute.
  - Replicated epilogue: nll = -(cl_sel - lse_cl + tgt - log(S_sel)).

Token layout on chip: token t -> (partition p = t % 128, tile i = t // 128).
"""

import os
import sys
from contextlib import ExitStack

import numpy as np

try:
    import concourse  # noqa: F401
except ImportError:  # pragma: no cover
    for _p in ("/opt/trn_rl_repo", "/root/.axon_site/_ro/trn_rl_repo"):
        if os.path.isdir(_p):
            sys.path.insert(0, _p)
            break

import ml_dtypes

import concourse.bass as bass
import concourse.tile as tile
from concourse import bacc, mybir
from concourse.bass_utils import run_bass_kernel_spmd

BF16 = ml_dtypes.bfloat16
FP8 = ml_dtypes.float8_e4m3

VOCAB, HIDDEN = 50257, 1024
NTOK = 4096          # B * L tokens
NCORES = 8
P = 128
NT = NTOK // P       # 32 token tiles
NTH = NT // 2        # 16 tiles per all-reduce half
B0, B1 = 250, 1250                 # shard-local cluster boundaries
SHARD = 250 + 1000 + 5032 + 1      # 6283 (incl shared col 50256)
WPAD = 6288                        # fp8 W free dim padded to %16
K2 = HIDDEN // 256                 # 4 double-row K chunks
LN8 = float(np.log(8.0))
SX, SW = 16.0, 64.0                # fp8 pre-scales for x and w
INV = 1.0 / (SX * SW)

# column groups (program order; big group last so AR#1 hides under it).
# group 0 computes 3 extra columns (6283:6286 in the padded W8) that hold
# the cluster-head weights; they are excluded from the exp segments.
GROUPS = [(6144, 6286), (0, 2048), (2048, 4096), (4096, 6144)]
# exp/accumulate segments outside group 0: (lo, hi, acc_col, biased)
BODY_SEGS = [
    (0, 250, 0, False),
    (250, 1250, 1, False),
    (1250, 2048, 2, False),
    (2048, 4096, 3, False),
    (4096, 6144, 4, False),
]
NSEG = 7


def _bank_subs(lo, hi):
    # split [lo, hi) at 512-col PSUM bank boundaries
    out = []
    c = lo
    while c < hi:
        nxt = min(hi, (c // 512 + 1) * 512)
        out.append((c, nxt))
        c = nxt
    return out


def _plan(cls, g):
    # Matmul sub-ranges + exp segments for a token tile of class cls
    # (0/1/2 = pure cluster, 3 = mixed) in column group g. Pure tiles only
    # compute their own cluster's columns (plus the 3 cluster-head pad cols
    # in group 0); the masked select in the epilogue ignores the rest.
    glo, ghi = GROUPS[g]
    if g == 0:
        if cls in (2, 3):
            return [(6144, 6286)], [(6144, 6282, 5, False), (6282, 6283, 6, True)]
        return [(6283, 6286)], []
    spans = {0: (0, 250), 1: (250, 1250), 2: (1250, 6144), 3: (0, 6144)}
    lo, hi = spans[cls]
    lo, hi = max(lo, glo), min(hi, ghi)
    if lo >= hi:
        return [], []
    segs = [(a, b, col, bia) for (a, b, col, bia) in BODY_SEGS if a >= lo and b <= hi]
    return _bank_subs(lo, hi), segs

LAST_RESULT = None  # BassKernelResults of the most recent run (side channel)


def _ensure_ntff_hook():
    """bass_utils' trace path imports antenv.axon_hooks, which the trimmed
    agent image lacks. Register a shim (ctypes NTFF hook if available, else
    None so tracing is skipped gracefully)."""
    try:
        import antenv.axon_hooks  # noqa: F401
        return
    except ImportError:
        pass
    hook = None
    try:
        if "/root/.axon_site" not in sys.path and os.path.isdir("/root/.axon_site"):
            sys.path.append("/root/.axon_site")
        from trn_agent_boot.trn_boot import _ntff_profile_via_ctypes
        hook = _ntff_profile_via_ctypes("/opt/axon/libaxon_pjrt.so")
    except Exception:
        hook = None
    import types

    import antenv

    m = types.ModuleType("antenv.axon_hooks")
    m.get_axon_ntff_profile_hook = lambda _hook=hook: _hook
    m.set_axon_ntff_profile_hook = lambda h: None
    sys.modules["antenv.axon_hooks"] = m
    antenv.axon_hooks = m


def _build_graph(kc, tile_classes):
    """Build the SPMD Bass graph. kc = number of 128-row K chunks.
    tile_classes[i] in {0,1,2,3}: cluster of sorted token tile i (3=mixed)."""
    assert kc % 2 == 0
    k2n = kc // 2
    hp = kc * P
    nc = bacc.Bacc(
        "TRN2",
        target_bir_lowering=False,
        debug=False,
        enable_asserts=False,
        num_devices=NCORES,
    )
    dt = mybir.dt
    fp = dt.float32
    f8 = dt.float8e4
    Exp = mybir.ActivationFunctionType.Exp
    Ln = mybir.ActivationFunctionType.Ln
    Alu = mybir.AluOpType
    X = mybir.AxisListType.X

    XT8 = nc.declare_dram_parameter("xt8", [P, k2n, 2, NTOK], f8, isOutput=False)
    W8 = nc.declare_dram_parameter("w8", [P, k2n, 2, WPAD], f8, isOutput=False)
    xN = nc.declare_dram_parameter("xn", [NTOK, hp], dt.bfloat16, isOutput=False)
    WT = nc.declare_dram_parameter("wt", [SHARD, hp], dt.bfloat16, isOutput=False)
    YI = nc.declare_dram_parameter("yi", [P, NT], dt.int32, isOutput=False)
    OM = nc.declare_dram_parameter("om", [P, NT], fp, isOutput=False)
    OH = nc.declare_dram_parameter("oh", [P, NT * 3], fp, isOutput=False)
    OUT = nc.declare_dram_parameter("out", [P, NT], fp, isOutput=True)

    plans = [[_plan(tile_classes[i], g) for g in range(len(GROUPS))]
             for i in range(NT)]

    with ExitStack() as ctx:
        tc = ctx.enter_context(tile.TileContext(nc))
        const = ctx.enter_context(tc.tile_pool(name="const", bufs=1))
        wpool = ctx.enter_context(tc.tile_pool(name="wpool", bufs=2))
        expp = ctx.enter_context(tc.tile_pool(name="expp", bufs=3))
        gpool = ctx.enter_context(tc.tile_pool(name="gpool", bufs=2))
        epi = ctx.enter_context(tc.tile_pool(name="epi", bufs=1))
        dram = ctx.enter_context(tc.tile_pool(name="dram", bufs=1, space="DRAM"))

        # ---- resident inputs ----
        xT_sb = const.tile([P, k2n, 2, NTOK], f8)

        def load_xt8_block(b):
            lo, hi = b * 1024, (b + 1) * 1024
            nc.sync.dma_start(
                out=xT_sb[:, :, :, lo:hi], in_=XT8[:, :, :, lo:hi]
            )

        load_xt8_block(0)
        yi_sb = const.tile([P, NT], dt.int32)
        nc.sync.dma_start(out=yi_sb[:], in_=YI[:, :])
        om_sb = const.tile([P, NT], fp)
        nc.sync.dma_start(out=om_sb[:], in_=OM[:, :])
        oh_sb = const.tile([P, NT * 3], fp)
        nc.sync.dma_start(out=oh_sb[:], in_=OH[:, :])

        nln8 = const.tile([P, 1], fp)
        nc.vector.memset(nln8[:], -LN8)

        acc = const.tile([P, NT * NSEG], fp)
        nc.vector.memset(acc[:], 0.0)
        tgt_raw = const.tile([P, NT], fp)
        # S_all layout: [half, quantity(S0,S1,S2,tgt), 16 tiles]
        S_all = const.tile([P, 2, 4, NTH], fp)
        R_all = const.tile([P, 2, 4, NTH], fp)
        cl_sb = const.tile([P, NT * 3], fp)

        # ---- target-logit path: gather owned weight rows, fused dot ----
        # (emitted mid main-loop so its DMA traffic doesn't block W8 loads)
        def emit_gather_block():
            for i in range(NT):
                wg = gpool.tile([P, hp], dt.bfloat16, tag="wg", name="wg")
                nc.gpsimd.indirect_dma_start(
                    out=wg[:],
                    out_offset=None,
                    in_=WT[:, :],
                    in_offset=bass.IndirectOffsetOnAxis(ap=yi_sb[:, i:i + 1], axis=0),
                )
                xr = gpool.tile([P, hp], dt.bfloat16, tag="xr", name="xr")
                nc.sync.dma_start(out=xr[:], in_=xN[i * P:(i + 1) * P, :])
                pr = gpool.tile([P, hp], fp, tag="pr", name="pr")
                nc.vector.tensor_mul(out=pr[:], in0=xr[:], in1=wg[:])
                nc.vector.reduce_sum(out=tgt_raw[:, i:i + 1], in_=pr[:], axis=X)

        # ---- main fp8 double-row matmul + fused exp/accumulate ----
        psum = ctx.enter_context(tc.tile_pool(name="psum", bufs=2, space="PSUM"))
        b_in = [
            dram.tile([P, 4 * NTH], fp, name=f"b_in{h}", tag=f"b_in{h}")
            for h in range(2)
        ]
        b_out = [
            dram.tile([P, 4 * NTH], fp, name=f"b_out{h}", tag=f"b_out{h}")
            for h in range(2)
        ]

        def reduce_half(h):
            """Fold acc + tgt partials for token-tile half h and start its
            all-reduce."""
            acc3 = acc[:].rearrange("p (i s) -> p i s", s=NSEG)
            sl = slice(h * NTH, (h + 1) * NTH)
            nc.vector.tensor_copy(out=S_all[:, h, 0, :], in_=acc3[:, sl, 0])
            nc.vector.tensor_copy(out=S_all[:, h, 1, :], in_=acc3[:, sl, 1])
            nc.vector.reduce_sum(out=S_all[:, h, 2, :], in_=acc3[:, sl, 2:NSEG], axis=X)
            nc.vector.tensor_mul(
                out=S_all[:, h, 3, :], in0=tgt_raw[:, sl], in1=om_sb[:, sl]
            )
            nc.gpsimd.dma_start(out=b_in[h][:], in_=S_all[:, h, :, :])
            nc.gpsimd.collective_compute(
                "AllReduce",
                Alu.add,
                replica_groups=[list(range(NCORES))],
                ins=[b_in[h].opt()],
                outs=[b_out[h].opt()],
            )
            nc.gpsimd.dma_start(out=R_all[:, h, :, :], in_=b_out[h][:])

        # ---- epilogue, split so only the AR-dependent suffix is on the
        # critical tail: cl_part = cl_sel - lse_cl precomputes after group 0.
        cl_part = epi.tile([P, NT], fp)

        def emit_cl_part():
            ecl = epi.tile([P, NT * 3], fp)
            nc.scalar.activation(out=ecl[:], in_=cl_sb[:], func=Exp)
            sum_cl = epi.tile([P, NT], fp)
            nc.vector.reduce_sum(
                out=sum_cl[:], in_=ecl[:].rearrange("p (i c) -> p i c", c=3), axis=X
            )
            lse_cl = epi.tile([P, NT], fp)
            nc.scalar.activation(out=lse_cl[:], in_=sum_cl[:], func=Ln)
            clsel_t = epi.tile([P, NT * 3], fp)
            nc.vector.tensor_mul(out=clsel_t[:], in0=cl_sb[:], in1=oh_sb[:])
            cl_sel = epi.tile([P, NT], fp)
            nc.vector.reduce_sum(
                out=cl_sel[:], in_=clsel_t[:].rearrange("p (i c) -> p i c", c=3),
                axis=X,
            )
            nc.vector.tensor_sub(out=cl_part[:], in0=cl_sel[:], in1=lse_cl[:])

        def emit_epilogue(h):
            hsl = slice(h * NTH, (h + 1) * NTH)      # [P, 16] ranges
            h3 = slice(h * NTH * 3, (h + 1) * NTH * 3)
            # R_all[:, h] is [P, 4, NTH]: S_c at [:, c, il]; view as [p, il, c]
            ssel_t = epi.tile([P, NTH * 3], fp, tag=f"ssel{h}", name=f"ssel{h}")
            rview = R_all[:, h, :, :].rearrange("p c il -> p il c")[:, :, 0:3]
            nc.vector.tensor_tensor(
                out=ssel_t[:].rearrange("p (il c) -> p il c", c=3),
                in0=rview,
                in1=oh_sb[:, h3].rearrange("p (il c) -> p il c", c=3),
                op=Alu.mult,
            )
            S_sel = epi.tile([P, NTH], fp, tag=f"S_sel{h}", name=f"S_sel{h}")
            nc.vector.reduce_sum(
                out=S_sel[:], in_=ssel_t[:].rearrange("p (i c) -> p i c", c=3), axis=X
            )
            logS = epi.tile([P, NTH], fp, tag=f"logS{h}", name=f"logS{h}")
            nc.scalar.activation(out=logS[:], in_=S_sel[:], func=Ln)
            t2 = epi.tile([P, NTH], fp, tag=f"t2{h}", name=f"t2{h}")
            nc.vector.tensor_sub(out=t2[:], in0=R_all[:, h, 3, :], in1=logS[:])
            # res = -(cl_part + t2) = (t2 * -1) - cl_part
            res = epi.tile([P, NTH], fp, tag=f"res{h}", name=f"res{h}")
            nc.vector.scalar_tensor_tensor(
                out=res[:], in0=t2[:], scalar=-1.0, in1=cl_part[:, hsl],
                op0=Alu.mult, op1=Alu.subtract,
            )
            nc.sync.dma_start(out=OUT[:, hsl], in_=res[:])

        n_groups = len(GROUPS)
        for g, (g0, g1) in enumerate(GROUPS):
            gw = g1 - g0
            wt_t = wpool.tile([P, k2n, 2, 2048], f8, tag="w")
            nc.sync.dma_start(
                out=wt_t[:, :, :, :gw], in_=W8[:, :, :, g0:g0 + gw]
            )
            if g == 0:
                for b in range(1, 4):
                    load_xt8_block(b)
            for i in range(NT):
                mm_subs, segs = plans[i][g]
                if mm_subs:
                    ps = psum.tile([P, 2048], fp)
                    for (slo, shi) in mm_subs:
                        for k in range(k2n):
                            nc.tensor.matmul(
                                ps[:, slo - g0:shi - g0],
                                lhsT=xT_sb[:, k, :, i * P:(i + 1) * P],
                                rhs=wt_t[:, k, :, slo - g0:shi - g0],
                                start=(k == 0),
                                stop=(k == k2n - 1),
                                perf_mode=mybir.MatmulPerfMode.DoubleRow,
                            )
                    if g == 0:
                        # cluster-head logits live in the 3 pad columns
                        nc.vector.tensor_scalar_mul(
                            cl_sb[:, i * 3:(i + 1) * 3], ps[:, 139:142], INV
                        )
                    ex = expp.tile([P, 2048], fp, tag="ex")
                    for (lo, hi, acc_col, biased) in segs:
                        nc.scalar.activation(
                            out=ex[:, lo - g0:hi - g0],
                            in_=ps[:, lo - g0:hi - g0],
                            func=Exp,
                            bias=(nln8[:] if biased else 0.0),
                            scale=INV,
                            accum_out=acc[:, i * NSEG + acc_col:i * NSEG + acc_col + 1],
                        )
                if g == n_groups - 1 and i == NTH - 1:
                    reduce_half(0)
                    emit_epilogue(0)
            if g == 0:
                emit_cl_part()
            if g == 2:
                emit_gather_block()
            if g == n_groups - 1:
                reduce_half(1)
                emit_epilogue(1)

    return nc


def _shard_cols(k):
    return np.concatenate(
        [
            np.arange(250 * k, 250 * (k + 1)),
            np.arange(2000 + 1000 * k, 2000 + 1000 * (k + 1)),
            np.arange(10000 + 5032 * k, 10000 + 5032 * (k + 1)),
            np.array([50256]),
        ]
    )


def _tok_layout(v):
    """[4096] vector -> [128, 32] with A[p, i] = v[i*128 + p]."""
    return np.ascontiguousarray(v.reshape(NT, P).T)


def _pack_dr(m, width):
    """[hp, width] -> double-row packed [128, hp//256, 2, width] fp8."""
    hp = m.shape[0]
    return np.ascontiguousarray(
        m.reshape(hp // 256, 2, P, width).transpose(2, 0, 1, 3)
    ).astype(FP8)


def kernel(**inputs):
    global LAST_RESULT
    x = np.asarray(inputs["x"], np.float32)
    y = np.asarray(inputs["y"]).astype(np.int64).reshape(-1)
    cw = np.asarray(inputs["cluster_w"], np.float32)
    cb = np.asarray(inputs["cluster_b"], np.float32).reshape(-1)
    lw = np.asarray(inputs["logits_w"], np.float32)
    lb = np.asarray(inputs["logits_b"], np.float32).reshape(-1)

    x_flat = x[:, :-1].reshape(NTOK, HIDDEN)

    # sort tokens by cluster so each 128-token tile is (mostly) one cluster;
    # pure tiles then only compute their own cluster's vocab columns.
    c_id_full = (y >= 2000).astype(np.int64) + (y >= 10000).astype(np.int64)
    order = np.argsort(c_id_full, kind="stable")
    x_flat = np.ascontiguousarray(x_flat[order])
    y = y[order]

    nz_bias = bool(np.any(cb)) or bool(np.any(lb))
    kc = HIDDEN // P + (2 if nz_bias else 0)
    hp = kc * P
    if nz_bias:
        # Fold biases in as extra hidden chunks (2 chunks to keep kc even):
        # x gets a column of ones (rest zeros), weights get the bias row.
        xa = np.zeros((NTOK, hp), np.float32)
        xa[:, :HIDDEN] = x_flat
        xa[:, HIDDEN] = 1.0
        lwa = np.zeros((hp, VOCAB), np.float32)
        lwa[:HIDDEN] = lw
        lwa[HIDDEN] = lb
        cwa = np.zeros((hp, 3), np.float32)
        cwa[:HIDDEN] = cw
        cwa[HIDDEN] = cb
        x_flat, lw, cw = xa, lwa, cwa

    xT = np.ascontiguousarray(x_flat.T)  # [hp, NTOK]
    xt8 = _pack_dr(xT * SX, NTOK)
    xN_bf = x_flat.astype(BF16)

    c_id = c_id_full[order]
    tile_classes = tuple(
        int(c_id[i * P]) if c_id[i * P] == c_id[(i + 1) * P - 1] else 3
        for i in range(NT)
    )
    # onehot over clusters, [128, 32*3] with c contiguous
    oh = np.zeros((NTOK, 3), np.float32)
    oh[np.arange(NTOK), c_id] = 1.0
    oh = np.ascontiguousarray(oh.reshape(NT, P, 3).transpose(1, 0, 2).reshape(P, NT * 3))

    in_maps = []
    for k in range(NCORES):
        cols = _shard_cols(k)
        w_sh = lw[:, cols]  # [hp, SHARD] f32
        wpadded = np.zeros((hp, WPAD), np.float32)
        wpadded[:, :SHARD] = w_sh
        wpadded[:, SHARD:SHARD + 3] = cw
        w8 = _pack_dr(wpadded * SW, WPAD)
        wt_bf = np.ascontiguousarray(w_sh.T).astype(BF16)

        loc = np.zeros(NTOK, np.int64)
        r0 = (y >= 250 * k) & (y < 250 * (k + 1))
        loc[r0] = y[r0] - 250 * k
        r1 = (y >= 2000 + 1000 * k) & (y < 2000 + 1000 * (k + 1))
        loc[r1] = 250 + y[r1] - (2000 + 1000 * k)
        r2 = (y >= 10000 + 5032 * k) & (y < 10000 + 5032 * (k + 1))
        loc[r2] = 1250 + y[r2] - (10000 + 5032 * k)
        own = r0 | r1 | r2
        if k == NCORES - 1:
            r3 = y == VOCAB - 1
            own = own | r3
            loc[r3] = SHARD - 1

        in_maps.append(
            {
                "xt8": xt8,
                "w8": w8,
                        "xn": xN_bf,
                "wt": wt_bf,
                "yi": _tok_layout(loc).astype(np.int32),
                "om": _tok_layout(own.astype(np.float32)),
                "oh": oh,
            }
        )

    _ensure_ntff_hook()
    nc = _build_graph(kc, tile_classes)
    if not nc.is_finalized():
        nc.finalize()  # bass2jax serializes as-is; Bacc needs alloc_regs etc.
    result = run_bass_kernel_spmd(nc, in_maps, core_ids=list(range(NCORES)))
    LAST_RESULT = result
    out = np.asarray(result.results[0]["out"], np.float32)  # [128, 32]
    nll_sorted = np.ascontiguousarray(out.T).reshape(-1)
    nll = np.empty(NTOK, np.float32)
    nll[order] = nll_sorted
    return nll



# revision 9
# speedup vs baseline: 1.0028x; 1.0028x over previous
"""Adaptive-softmax NLL loss kernel for 8 TRN2 NeuronCores.

Strategy (vocab-parallel tensor parallelism + cluster-sorted tokens),
restructured tile-outer for collective overlap:
  - Each core owns a 1/8 column slice of each cluster's vocab range
    (250 + 1000 + 5032 cols) plus the shared remainder column 50256
    (its exp is scaled by 1/8 on every core so the reduced sum is exact).
  - The per-core weight matrix is laid out so every tile's needed columns
    are ONE contiguous span: [c0 250 | heads 3 | c1 1000 | heads 3 |
    c2 5032 | shared 1].  The 3 cluster-head columns are duplicated so
    both light (c0/c1) and heavy (c2) tiles pick them up inside their
    span, with exp segments skipping them.
  - Tokens are host-sorted by cluster id so each 128-token tile is
    (almost always) single-cluster; the output is unscrambled on host.
  - Processing is TILE-OUTER: all weights stay resident in SBUF; each
    tile marches through its span in <=2048-col PSUM chunks (4 banks,
    double buffered).  fp8e4m3 DoubleRow matmuls with k-outer ordering
    (stationary x reused across column sub-blocks).
  - ScalarE computes exp over chunk spans with fused free-dim
    accumulation into 4 fixed slots per tile; mixed-cluster tiles get
    spare slots + a onehot fixup.
  - Target logit x[t] . w[y_t] via indirect-DMA gather of owned weight
    rows (bf16) + multiply/reduce on VectorE, masked by ownership.
  - The cross-core combine is 4 chunked 8KB AllReduces of
    (S_selected, tgt) issued as each 8-tile quarter completes, so only
    the last small AR is exposed in the tail.
  - All Ln/epilogue work is deferred to after the last AR so the Scalar
    FIFO never blocks the exp stream (exactly one Exp->Ln table swap).

Token layout on chip: token t -> (partition p = t % 128, tile i = t // 128).
"""

import os
import sys
from contextlib import ExitStack

import numpy as np

try:
    import concourse  # noqa: F401
except ImportError:  # pragma: no cover
    for _p in ("/opt/trn_rl_repo", "/root/.axon_site/_ro/trn_rl_repo"):
        if os.path.isdir(_p):
            sys.path.insert(0, _p)
            break

import ml_dtypes

import concourse.bass as bass
import concourse.tile as tile
from concourse import bacc, mybir
from concourse.bass_utils import run_bass_kernel_spmd

BF16 = ml_dtypes.bfloat16
FP8 = ml_dtypes.float8_e4m3

VOCAB, HIDDEN = 50257, 1024
NTOK = 4096          # B * L tokens
NCORES = 8
P = 128
NT = NTOK // P       # 32 token tiles
NQ = 4               # AllReduce chunks
QT = NT // NQ        # 8 tiles per chunk
LN8 = float(np.log(8.0))
SX, SW = 16.0, 64.0  # fp8 pre-scales for x and w
INV = 1.0 / (SX * SW)

# --- per-core column geometry (shard-local) ---
C0N, C1N, C2N = 250, 1000, 5032
HA0, HA1 = 250, 253             # cluster-head copy A
C1S, C1E = 253, 1253
HB0, HB1 = 1253, 1256           # cluster-head copy B
C2S, C2E = 1256, 6288
SH = 6288                       # shared remainder column 50256
NCOL = 6289
WPAD = 6304                     # fp8 W free dim padded to %16
SHARD = C0N + C1N + C2N + 1     # 6283 rows in gather table

CLUSTER_SEG = {0: (0, C0N), 1: (C1S, C1E), 2: (C2S, C2E)}
SPAN_LO = {0: 0, 1: HA0, 2: HB0}
SPAN_HI = {0: HA1, 1: C1E, 2: NCOL}
HEADS_AT = {0: (HA0, HA1), 1: (HA0, HA1), 2: (HB0, HB1)}

CHUNK = 2048                    # PSUM window (4 banks)


def _tile_plan(clusters):
    """Static plan for one tile given its sorted cluster list.

    Returns dict with span, chunk windows, per-chunk matmul subs and exp
    segments.  Segments carry (lo, hi, cluster, biased).  Slot ids are
    assigned later (main cluster = highest cluster id present)."""
    lo = SPAN_LO[clusters[0]]
    hi = SPAN_HI[clusters[-1]]
    heads = HEADS_AT[clusters[0]]
    segs = []
    for c in clusters:
        segs.append((CLUSTER_SEG[c][0], CLUSTER_SEG[c][1], c, False))
    if clusters[-1] == 2:
        segs.append((SH, SH + 1, 2, True))
    windows = []
    w = lo
    while w < hi:
        w1 = min(hi, w + CHUNK)
        # bank subs relative to window start
        subs = []
        c = w
        while c < w1:
            nxt = min(w1, c + 512 - (c - w) % 512)
            subs.append((c, nxt))
            c = nxt
        csegs = []
        for (a, b, cl, bia) in segs:
            aa, bb = max(a, w), min(b, w1)
            if aa < bb:
                csegs.append((aa, bb, cl, bia))
        windows.append({"w": (w, w1), "subs": subs, "segs": csegs})
        w = w1
    main = clusters[-1]
    return {"span": (lo, hi), "heads": heads, "windows": windows,
            "clusters": clusters, "main": main}


LAST_RESULT = None  # BassKernelResults of the most recent run (side channel)


def _ensure_ntff_hook():
    """bass_utils' trace path imports antenv.axon_hooks, which the trimmed
    agent image lacks. Register a shim (ctypes NTFF hook if available, else
    None so tracing is skipped gracefully)."""
    try:
        import antenv.axon_hooks  # noqa: F401
        return
    except ImportError:
        pass
    hook = None
    try:
        if "/root/.axon_site" not in sys.path and os.path.isdir("/root/.axon_site"):
            sys.path.append("/root/.axon_site")
        from trn_agent_boot.trn_boot import _ntff_profile_via_ctypes
        hook = _ntff_profile_via_ctypes("/opt/axon/libaxon_pjrt.so")
    except Exception:
        hook = None
    import types

    import antenv

    m = types.ModuleType("antenv.axon_hooks")
    m.get_axon_ntff_profile_hook = lambda _hook=hook: _hook
    m.set_axon_ntff_profile_hook = lambda h: None
    sys.modules["antenv.axon_hooks"] = m
    antenv.axon_hooks = m


def _build_graph(kc, plans, order_proc):
    """Build the SPMD Bass graph. kc = number of 128-row K chunks.
    plans[i] = _tile_plan for original tile i; order_proc = processing
    order of tile indices (heavy c2 first, mixed last)."""
    assert kc % 2 == 0
    k2n = kc // 2
    hp = kc * P
    nc = bacc.Bacc(
        "TRN2",
        target_bir_lowering=False,
        debug=False,
        enable_asserts=False,
        num_devices=NCORES,
    )
    dt = mybir.dt
    fp = dt.float32
    f8 = dt.float8e4
    Exp = mybir.ActivationFunctionType.Exp
    Ln = mybir.ActivationFunctionType.Ln
    Alu = mybir.AluOpType
    X = mybir.AxisListType.X

    XT8 = nc.declare_dram_parameter("xt8", [P, k2n, 2, NTOK], f8, isOutput=False)
    W8 = nc.declare_dram_parameter("w8", [P, k2n, 2, WPAD], f8, isOutput=False)
    xN = nc.declare_dram_parameter("xn", [NTOK, hp], dt.bfloat16, isOutput=False)
    WT = nc.declare_dram_parameter("wt", [SHARD, hp], dt.bfloat16, isOutput=False)
    YI = nc.declare_dram_parameter("yi", [P, NT], dt.int32, isOutput=False)
    OM = nc.declare_dram_parameter("om", [P, NT], fp, isOutput=False)
    OH = nc.declare_dram_parameter("oh", [P, NT * 3], fp, isOutput=False)
    OUT = nc.declare_dram_parameter("out", [P, NT], fp, isOutput=True)

    # spare-slot assignment for mixed tiles: list of (proc_pos, cluster, nsegs)
    nspare = 0
    spare_of = {}  # proc_pos -> (offset, cluster, count)
    for pos, t in enumerate(order_proc):
        pl = plans[t]
        if len(pl["clusters"]) > 1:
            # all clusters except main go to spare slots
            cnt = 0
            for wnd in pl["windows"]:
                for (a, b, cl, bia) in wnd["segs"]:
                    if cl != pl["main"]:
                        cnt += 1
            assert len(pl["clusters"]) == 2, "only 2-cluster mixed tiles supported"
            spare_of[pos] = (nspare, pl["clusters"][0], cnt)
            nspare += cnt
    nspare = max(nspare, 1)

    with ExitStack() as ctx:
        tc = ctx.enter_context(tile.TileContext(nc))
        const = ctx.enter_context(tc.tile_pool(name="const", bufs=1))
        expp = ctx.enter_context(tc.tile_pool(name="expp", bufs=3))
        gpool = ctx.enter_context(tc.tile_pool(name="gpool", bufs=2))
        epi = ctx.enter_context(tc.tile_pool(name="epi", bufs=1))
        dram = ctx.enter_context(tc.tile_pool(name="dram", bufs=1, space="DRAM"))

        # ---- resident inputs ----
        yi_sb = const.tile([P, NT], dt.int32)
        nc.sync.dma_start(out=yi_sb[:], in_=YI[:, :])
        om_sb = const.tile([P, NT], fp)
        nc.sync.dma_start(out=om_sb[:], in_=OM[:, :])
        oh_sb = const.tile([P, NT * 3], fp)
        nc.sync.dma_start(out=oh_sb[:], in_=OH[:, :])

        w8_sb = const.tile([P, k2n, 2, WPAD], f8)
        xT_sb = const.tile([P, k2n, 2, NTOK], f8)

        # first-needed pieces first: first processed tile is a c2 tile whose
        # span starts at HB0; its x columns sit in xt8 block 0 or 1.
        first_tile = order_proc[0]
        fx_blk = (first_tile * P) // 1024
        nc.sync.dma_start(
            out=w8_sb[:, :, :, HB0:HB0 + 1024], in_=W8[:, :, :, HB0:HB0 + 1024]
        )

        def load_xt8_block(b):
            lo, hi = b * 1024, (b + 1) * 1024
            nc.sync.dma_start(out=xT_sb[:, :, :, lo:hi], in_=XT8[:, :, :, lo:hi])

        load_xt8_block(fx_blk)
        nc.sync.dma_start(
            out=w8_sb[:, :, :, HB0 + 1024:HB0 + 3072],
            in_=W8[:, :, :, HB0 + 1024:HB0 + 3072],
        )
        for b in range(4):
            if b != fx_blk:
                load_xt8_block(b)
        nc.sync.dma_start(
            out=w8_sb[:, :, :, HB0 + 3072:WPAD], in_=W8[:, :, :, HB0 + 3072:WPAD]
        )
        nc.sync.dma_start(out=w8_sb[:, :, :, 0:HB0], in_=W8[:, :, :, 0:HB0])

        nln8 = const.tile([P, 1], fp)
        nc.vector.memset(nln8[:], -LN8)

        acc = const.tile([P, NT * 4], fp)      # 4 main slots per tile (proc order)
        nc.vector.memset(acc[:], 0.0)
        accs = const.tile([P, nspare], fp)     # spare slots for mixed tiles
        nc.vector.memset(accs[:], 0.0)
        tgt_raw = const.tile([P, NT], fp)      # proc order
        cl_sb = const.tile([P, NT * 3], fp)    # ORIGINAL tile order (for oh)
        # S_all layout: [q, (S, tgt), QT tiles]
        S_all = const.tile([P, NQ, 2, QT], fp)
        R_all = const.tile([P, NQ, 2, QT], fp)

        psum = ctx.enter_context(tc.tile_pool(name="psum", bufs=2, space="PSUM"))
        b_in = [dram.tile([P, 2 * QT], fp, name=f"b_in{q}", tag=f"b_in{q}")
                for q in range(NQ)]
        b_out = [dram.tile([P, 2 * QT], fp, name=f"b_out{q}", tag=f"b_out{q}")
                 for q in range(NQ)]

        def emit_tile(pos, t):
            pl = plans[t]
            slot = 0
            sp_off = spare_of.get(pos, (0, -1, 0))[0]
            heads_done = False
            for wnd in pl["windows"]:
                w0, w1 = wnd["w"]
                ps = psum.tile([P, CHUNK], fp)
                for k in range(k2n):
                    for (a, b) in wnd["subs"]:
                        nc.tensor.matmul(
                            ps[:, a - w0:b - w0],
                            lhsT=xT_sb[:, k, :, t * P:(t + 1) * P],
                            rhs=w8_sb[:, k, :, a:b],
                            start=(k == 0),
                            stop=(k == k2n - 1),
                            perf_mode=mybir.MatmulPerfMode.DoubleRow,
                        )
                if not heads_done and pl["heads"][0] >= w0 and pl["heads"][1] <= w1:
                    h0, h1 = pl["heads"]
                    nc.vector.tensor_scalar_mul(
                        cl_sb[:, pos * 3:(pos + 1) * 3], ps[:, h0 - w0:h1 - w0], INV
                    )
                    heads_done = True
                ex = expp.tile([P, CHUNK], fp, tag="ex")
                for (a, b, cl, bia) in wnd["segs"]:
                    if cl == pl["main"]:
                        acol = acc[:, pos * 4 + slot:pos * 4 + slot + 1]
                        slot += 1
                    else:
                        acol = accs[:, sp_off:sp_off + 1]
                        sp_off += 1
                    nc.scalar.activation(
                        out=ex[:, a - w0:b - w0],
                        in_=ps[:, a - w0:b - w0],
                        func=Exp,
                        bias=(nln8[:] if bia else 0.0),
                        scale=INV,
                        accum_out=acol,
                    )
            assert heads_done and slot <= 4
            # target-logit gather + dot (yi is proc-ordered on host)
            wg = gpool.tile([P, hp], dt.bfloat16, tag="wg", name="wg")
            nc.gpsimd.indirect_dma_start(
                out=wg[:],
                out_offset=None,
                in_=WT[:, :],
                in_offset=bass.IndirectOffsetOnAxis(ap=yi_sb[:, pos:pos + 1], axis=0),
            )
            xr = gpool.tile([P, hp], dt.bfloat16, tag="xr", name="xr")
            nc.sync.dma_start(out=xr[:], in_=xN[t * P:(t + 1) * P, :])
            pr = gpool.tile([P, hp], fp, tag="pr", name="pr")
            nc.vector.tensor_mul(out=pr[:], in0=xr[:], in1=wg[:])
            nc.vector.reduce_sum(out=tgt_raw[:, pos:pos + 1], in_=pr[:], axis=X)

        def emit_quarter(q):
            """Fold acc slots + tgt for proc positions [q*QT, (q+1)*QT) and
            start the chunk's all-reduce."""
            sl = slice(q * QT, (q + 1) * QT)
            acc4 = acc[:].rearrange("p (i s) -> p i s", s=4)
            nc.vector.reduce_sum(out=S_all[:, q, 0, :], in_=acc4[:, sl, :], axis=X)
            # mixed-tile fixup: S = S_main*oh_main + S_spare*oh_spare
            for pos in range(q * QT, (q + 1) * QT):
                if pos not in spare_of:
                    continue
                off, cl_sp, cnt = spare_of[pos]
                pl = plans[order_proc[pos]]
                j = pos - q * QT
                scol = S_all[:, q, 0, j:j + 1]
                nc.vector.tensor_mul(
                    out=scol, in0=scol,
                    in1=oh_sb[:, pos * 3 + pl["main"]:pos * 3 + pl["main"] + 1],
                )
                sps = epi.tile([P, 1], fp, tag=f"sps{pos}", name=f"sps{pos}")
                if cnt > 1:
                    nc.vector.reduce_sum(
                        out=sps[:], in_=accs[:, off:off + cnt], axis=X
                    )
                    src = sps[:]
                else:
                    src = accs[:, off:off + 1]
                nc.vector.tensor_mul(
                    out=sps[:], in0=src,
                    in1=oh_sb[:, pos * 3 + cl_sp:pos * 3 + cl_sp + 1],
                )
                nc.vector.tensor_tensor(out=scol, in0=scol, in1=sps[:], op=Alu.add)
            # tgt partial, ownership-masked (om is proc-ordered on host)
            nc.vector.tensor_mul(
                out=S_all[:, q, 1, :], in0=tgt_raw[:, sl], in1=om_sb[:, sl]
            )
            nc.gpsimd.dma_start(out=b_in[q][:], in_=S_all[:, q, :, :])
            nc.gpsimd.collective_compute(
                "AllReduce",
                Alu.add,
                replica_groups=[list(range(NCORES))],
                ins=[b_in[q].opt()],
                outs=[b_out[q].opt()],
            )

        for pos, t in enumerate(order_proc):
            emit_tile(pos, t)
            if (pos + 1) % QT == 0:
                emit_quarter((pos + 1) // QT - 1)

        # ---- cluster-head path (all tiles, original order) ----
        ecl = epi.tile([P, NT * 3], fp)
        nc.scalar.activation(out=ecl[:], in_=cl_sb[:], func=Exp)
        sum_cl = epi.tile([P, NT], fp)
        nc.vector.reduce_sum(
            out=sum_cl[:], in_=ecl[:].rearrange("p (i c) -> p i c", c=3), axis=X
        )
        clsel_t = epi.tile([P, NT * 3], fp)
        nc.vector.tensor_mul(out=clsel_t[:], in0=cl_sb[:], in1=oh_sb[:])
        cl_sel = epi.tile([P, NT], fp)
        nc.vector.reduce_sum(
            out=cl_sel[:], in_=clsel_t[:].rearrange("p (i c) -> p i c", c=3), axis=X
        )

        # ---- epilogue: everything below waits on collectives ----
        for q in range(NQ):
            nc.sync.dma_start(out=R_all[:, q, :, :], in_=b_out[q][:])
        logS = epi.tile([P, NT], fp)        # proc order
        nc.scalar.activation(
            out=logS[:].rearrange("p (q j) -> p q j", j=QT),
            in_=R_all[:, :, 0, :],
            func=Ln,
        )
        lse_cl = epi.tile([P, NT], fp)      # original order
        nc.scalar.activation(out=lse_cl[:], in_=sum_cl[:], func=Ln)
        # nll[pos] = logS - tgt - cl_sel + lse_cl  (orig-order cols via map)
        v1 = epi.tile([P, NT], fp)          # proc order
        nc.vector.tensor_tensor(
            out=v1[:].rearrange("p (q j) -> p q j", j=QT),
            in0=logS[:].rearrange("p (q j) -> p q j", j=QT),
            in1=R_all[:, :, 1, :],
            op=Alu.subtract,
        )
        v2 = epi.tile([P, NT], fp)          # proc order (cl/oh proc-ordered)
        nc.vector.tensor_sub(out=v2[:], in0=cl_sel[:], in1=lse_cl[:])
        res = epi.tile([P, NT], fp)         # proc order
        nc.vector.tensor_sub(out=res[:], in0=v1[:], in1=v2[:])
        nc.sync.dma_start(out=OUT[:, :], in_=res[:])

    return nc


def _shard_cols(k):
    c0 = np.arange(250 * k, 250 * (k + 1))
    c1 = np.arange(2000 + 1000 * k, 2000 + 1000 * (k + 1))
    c2 = np.arange(10000 + 5032 * k, 10000 + 5032 * (k + 1))
    return c0, c1, c2


def _tok_layout(v):
    """[4096] vector -> [128, 32] with A[p, i] = v[i*128 + p]."""
    return np.ascontiguousarray(v.reshape(NT, P).T)


def _pack_dr(m, width):
    """[hp, width] -> double-row packed [128, hp//256, 2, width] fp8."""
    hp = m.shape[0]
    return np.ascontiguousarray(
        m.reshape(hp // 256, 2, P, width).transpose(2, 0, 1, 3)
    ).astype(FP8)


def kernel(**inputs):
    global LAST_RESULT
    x = np.asarray(inputs["x"], np.float32)
    y = np.asarray(inputs["y"]).astype(np.int64).reshape(-1)
    cw = np.asarray(inputs["cluster_w"], np.float32)
    cb = np.asarray(inputs["cluster_b"], np.float32).reshape(-1)
    lw = np.asarray(inputs["logits_w"], np.float32)
    lb = np.asarray(inputs["logits_b"], np.float32).reshape(-1)

    x_flat = x[:, :-1].reshape(NTOK, HIDDEN)

    # sort tokens by cluster so each 128-token tile is (mostly) one cluster
    c_id_full = (y >= 2000).astype(np.int64) + (y >= 10000).astype(np.int64)
    order = np.argsort(c_id_full, kind="stable")
    x_flat = np.ascontiguousarray(x_flat[order])
    y = y[order]
    c_id = c_id_full[order]

    nz_bias = bool(np.any(cb)) or bool(np.any(lb))
    kc = HIDDEN // P + (2 if nz_bias else 0)
    hp = kc * P
    if nz_bias:
        xa = np.zeros((NTOK, hp), np.float32)
        xa[:, :HIDDEN] = x_flat
        xa[:, HIDDEN] = 1.0
        lwa = np.zeros((hp, VOCAB), np.float32)
        lwa[:HIDDEN] = lw
        lwa[HIDDEN] = lb
        cwa = np.zeros((hp, 3), np.float32)
        cwa[:HIDDEN] = cw
        cwa[HIDDEN] = cb
        x_flat, lw, cw = xa, lwa, cwa

    xT = np.ascontiguousarray(x_flat.T)  # [hp, NTOK]
    xt8 = _pack_dr(xT * SX, NTOK)
    xN_bf = x_flat.astype(BF16)

    # per-tile cluster lists + processing order: pure c2, light, mixed last
    tiles_cl = []
    for i in range(NT):
        tiles_cl.append(sorted(set(c_id[i * P:(i + 1) * P].tolist())))
    pure2 = [i for i in range(NT) if tiles_cl[i] == [2]]
    light = [i for i in range(NT) if tiles_cl[i] in ([0], [1])]
    mixed = [i for i in range(NT) if len(tiles_cl[i]) > 1]
    order_proc = pure2 + light + mixed
    assert len(order_proc) == NT
    plans = [_tile_plan(cl) for cl in tiles_cl]

    # onehot over clusters, [128, 32*3] with c contiguous, PROC tile order
    oh = np.zeros((NTOK, 3), np.float32)
    oh[np.arange(NTOK), c_id] = 1.0
    oh = oh.reshape(NT, P, 3)[order_proc]
    oh = np.ascontiguousarray(oh.transpose(1, 0, 2).reshape(P, NT * 3))

    in_maps = []
    for k in range(NCORES):
        c0, c1, c2 = _shard_cols(k)
        wpadded = np.zeros((hp, WPAD), np.float32)
        wpadded[:, 0:C0N] = lw[:, c0]
        wpadded[:, HA0:HA1] = cw
        wpadded[:, C1S:C1E] = lw[:, c1]
        wpadded[:, HB0:HB1] = cw
        wpadded[:, C2S:C2E] = lw[:, c2]
        wpadded[:, SH] = lw[:, VOCAB - 1]  # shared col (exp biased by -ln8)
        w8 = _pack_dr(wpadded * SW, WPAD)

        # gather table rows: [c0 | c1 | c2 | shared]
        w_sh = np.concatenate(
            [lw[:, c0], lw[:, c1], lw[:, c2], lw[:, VOCAB - 1:VOCAB]], axis=1)
        wt_bf = np.ascontiguousarray(w_sh.T).astype(BF16)

        loc = np.zeros(NTOK, np.int64)
        r0 = (y >= 250 * k) & (y < 250 * (k + 1))
        loc[r0] = y[r0] - 250 * k
        r1 = (y >= 2000 + 1000 * k) & (y < 2000 + 1000 * (k + 1))
        loc[r1] = 250 + y[r1] - (2000 + 1000 * k)
        r2 = (y >= 10000 + 5032 * k) & (y < 10000 + 5032 * (k + 1))
        loc[r2] = 1250 + y[r2] - (10000 + 5032 * k)
        own = r0 | r1 | r2
        if k == NCORES - 1:
            r3 = y == VOCAB - 1
            own = own | r3
            loc[r3] = SHARD - 1

        in_maps.append(
            {
                "xt8": xt8,
                "w8": w8,
                "xn": xN_bf,
                "wt": wt_bf,
                # yi/om in PROC tile order to match on-chip indexing
                "yi": np.ascontiguousarray(
                    _tok_layout(loc)[:, order_proc]).astype(np.int32),
                "om": np.ascontiguousarray(
                    _tok_layout(own.astype(np.float32))[:, order_proc]),
                "oh": oh,
            }
        )

    _ensure_ntff_hook()
    nc = _build_graph(kc, plans, order_proc)
    if not nc.is_finalized():
        nc.finalize()
    result = run_bass_kernel_spmd(nc, in_maps, core_ids=list(range(NCORES)))
    LAST_RESULT = result
    out = np.asarray(result.results[0]["out"], np.float32)  # [128, 32] proc order
    nll_sorted = np.empty(NTOK, np.float32)
    for pos, t in enumerate(order_proc):
        nll_sorted[t * P:(t + 1) * P] = out[:, pos]
    nll = np.empty(NTOK, np.float32)
    nll[order] = nll_sorted
    return nll
